# revision 54
# baseline (speedup 1.0000x reference)
"""BoxCrop kernel for Trainium2 (8 NeuronCores, Bass/Tile) — v3.

Fused crop -> aspect-preserving bilinear resize (long side 336) -> square pad
(fill=127) for a batch of 64 images [64,3,768,768] with per-image XYWH boxes.

Strategy (pure data-parallel, 8 images per core, single SPMD program):
- The host extracts each crop window, subtracts the fill value (127) and packs
  it as bf16 row-major [h, 3, w] into a per-core flat buffer using per-slot
  compile-time shapes (slot maxima over the 8 cores; an annealed clustering
  keeps the 8 images sharing a slot similar).  The device reads the packed
  bytes with static-offset DMAs (~half the f32 bytes, no dynamic bases).
  Because the data is offset by -127 and the tent weight matrices are exactly
  zero outside each image's valid output region, the final result is simply
  out = (crop' interpolated) + 127 everywhere — no fill masks.
- Separable bilinear resize as two matmul stages with *negated* tent-weight
  matrices built on-device from host-sent (bf16 hi/lo) source coordinates:
      stage1: rt[w, y]  = crop[h,w]^T @ tentY[h, y]      (contract over rows)
      stage2: m[y, x]   = rt[w, y-chunk]^T @ tentX[w, x]  (contract over cols)
  Matmuls cover only the narrow output regions each 128-row/col piece can
  influence (the coordinate map is monotone) within centered 128-aligned y/x
  windows; tent construction is region-limited too, with y/x piece pairs
  sharing one Abs (Act) + one min (DVE/Pool) op via a common iota bias.
- PSUM drains (rt copies, +127 output moves) may only run on Act/DVE (GPSIMD
  cannot touch PSUM); they are span-narrowed, with Pool memsets zero/fill
  padding the strips.  The +127 de-offset rides the mandatory move.
- All-127 y-bands outside the window are written straight from a constant
  SBUF tile by dedicated DMAs.  All DMAs ride the SP HWDGE queue in the order
  [consts, crops, bands, outs]; crops are all prefetched up-front (they fit
  in SBUF), so reads never stall behind compute-gated writes.  The output
  leaves the device y-major ([slot, y, c, x], contiguous 4KB descriptors);
  the host transposes back when unpacking.
"""
import numpy as np

import concourse.bacc as bacc
import concourse.tile as tile
from concourse import mybir
from concourse.bass import AP
from concourse.bass_utils import run_bass_kernel_spmd

F32 = mybir.dt.float32
BF16 = mybir.dt.bfloat16
NTAB = 6   # iota cols 0-3 (p+128t), col 4 = 127 bias, col 5 spare

N_CORES = 8
B = 64
BL = B // N_CORES          # images per core (= slots)
C = 3
H = W = 768
O = 336
FILL = 127.0

_CACHED = {}
LAST_RESULT = None

# schedule knobs (tuned against TimelineSim)
CFG = dict(PA=2, SA=2, st2_first=True, tent_bufs=6, dtmp_bufs=4, rt_bufs=4,
           osb_bufs=5, ps1_bufs=2, ps2_bufs=3, pair=True, pair1=False,
           slot_order="zigzag", ramp=0, const_after=-1, bands_late=1,
           split_out=1)

MARKS = []             # (inst_number, slot, phase) emission markers


def _mark(nc, slot, phase):
    MARKS.append((int(nc.get_next_instruction_name()[2:]), slot, phase))


# ---------------------------------------------------------------------------
# geometry helpers (must match reference semantics exactly)

def _img_geom(h, w):
    f = np.float32
    scale = f(O) / np.maximum(f(w), f(h))
    nw = int(np.round(f(w) * scale))
    nh = int(np.round(f(h) * scale))
    pad_top = (O - nh) // 2 if h < w else 0
    pad_left = (O - nw) // 2 if h >= w else 0
    return nh, nw, pad_top, pad_left


def _axis_coords(pad, new_n, n, w0, ww):
    """Crop-local source coords over window [w0, w0+ww); -1e6 where invalid."""
    f = np.float32
    i = np.arange(w0, w0 + ww, dtype=np.int64) - pad
    valid = (i >= 0) & (i < new_n)
    src = (i.astype(f) + f(0.5)) * f(n)
    src = src / f(new_n)
    src = src - f(0.5)
    src = np.clip(src, f(0.0), f(n - 1))
    src[~valid] = f(-1e6)
    return src.astype(np.float32), valid


def _windows(nvmax, nxmax):
    nv = -(-nvmax // 128)
    nx = -(-nxmax // 128)
    YW = min(128 * nv, O)
    XW = min(128 * nx, O)
    Y0 = (O - YW) // 2
    X0 = (O - XW) // 2
    cs = YW // nv
    return nv, nx, YW, XW, Y0, X0, cs


def _piece_intervals(src_list, n_pieces, n_rows, width):
    """True bounding interval per 128-row piece over all images (None if the
    piece influences nothing), plus matmul regions covering the union span."""
    ivs = []
    for t in range(n_pieces):
        a = 128 * t
        b = min(128 * (t + 1), n_rows)
        lo, hi = None, None
        for src in src_list:
            ys = np.nonzero((src >= a - 1) & (src <= b))[0]
            if len(ys):
                l, r = int(ys[0]), int(ys[-1]) + 1
                lo = l if lo is None else min(lo, l)
                hi = r if hi is None else max(hi, r)
        ivs.append(None if lo is None else [lo, hi])
    span_lo = min(iv[0] for iv in ivs if iv)
    span_hi = max(iv[1] for iv in ivs if iv)
    pts = {span_lo, span_hi}
    for iv in ivs:
        if iv:
            pts.update(p for p in iv if span_lo < p < span_hi)
    pts = sorted(pts)
    regions = []
    for e0, e1 in zip(pts[:-1], pts[1:]):
        cover = [t for t, iv in enumerate(ivs)
                 if iv and iv[0] <= e0 and iv[1] >= e1]
        if not cover:
            # interior gap (all-invalid outputs): nearest piece writes zeros
            cands = [t for t, iv in enumerate(ivs) if iv]
            t = min(cands, key=lambda t: min(abs(ivs[t][0] - e1),
                                             abs(ivs[t][1] - e0)))
            ivs[t][0] = min(ivs[t][0], e0)
            ivs[t][1] = max(ivs[t][1], e1)
            cover = [t]
        regions.append((e0, e1, tuple(cover)))
    merged = [list(regions[0])]
    for r in regions[1:]:
        if r[2] == merged[-1][2]:
            merged[-1][1] = r[1]
        else:
            merged.append(list(r))
    for t, iv in enumerate(ivs):
        if iv is None:
            ivs[t] = [0, 1]     # unused piece: harmless 1-wide block
    blocks = tuple((int(iv[0]), int(iv[1])) for iv in ivs)
    regions = tuple((int(r[0]), int(r[1]), r[2]) for r in merged)
    return blocks, regions, (int(span_lo), int(span_hi))


def _slot_spec(idx, boxes):
    hs = [int(boxes[i, 3]) for i in idx]
    ws = [int(boxes[i, 2]) for i in idx]
    RH = max(hs)
    RW = max(max(ws), 86)
    ch = -(-RH // 128)
    bw = -(-RW // 128)
    geo = [_img_geom(h, w) for h, w in zip(hs, ws)]
    nv, nx, YW, XW, Y0, X0, cs = _windows(max(g[0] for g in geo),
                                          max(g[1] for g in geo))
    sycs, sxcs = [], []
    for (h, w), (nh, nw, pt, pl) in zip(zip(hs, ws), geo):
        syc, _ = _axis_coords(pt, nh, h, Y0, YW)
        sxc, _ = _axis_coords(pl, nw, w, X0, XW)
        sycs.append(syc)
        sxcs.append(sxc)
    ybl, yreg, yspan = _piece_intervals(sycs, ch, RH, YW)
    xbl, xreg, xspan = _piece_intervals(sxcs, bw, RW, XW)
    # pair blocks t < m share one abs+min op over a joint window
    m = min(ch, bw)
    ybl = [list(b) for b in ybl]
    xbl = [list(b) for b in xbl]
    for t in range(m):
        jlo = min(ybl[t][0], xbl[t][0])
        jhi = max(ybl[t][1], xbl[t][1])
        ybl[t] = [jlo, jhi]
        xbl[t] = [jlo, jhi]
    ybl = tuple(tuple(b) for b in ybl)
    xbl = tuple(tuple(b) for b in xbl)
    return (RH, RW, ch, bw, nv, nx, YW, XW, Y0, X0, cs,
            ybl, yreg, yspan, xbl, xreg, xspan)


# ---------------------------------------------------------------------------
# clustering

def _plan(boxes: np.ndarray):
    h = boxes[:, 3].astype(np.int64)
    w = boxes[:, 2].astype(np.int64)
    scale = np.float64(O) / np.maximum(w, h)
    nh = np.round(h * scale).astype(np.int64)
    nw = np.round(w * scale).astype(np.int64)

    def proxy(g):
        RH = int(h[g].max())
        RW = max(int(w[g].max()), 86)
        ch = -(-RH // 128)
        bw = -(-RW // 128)
        nv, nx, YW, XW, _, _, _ = _windows(int(nh[g].max()), int(nw[g].max()))
        lat = 2.0 if 6 * RW < 512 else 1.0
        rows = 128 * ch if (128 * ch - RH) * 6 * RW <= 40000 else RH
        read = rows * 3 * RW * 2 * lat / 360.0
        ov = 40.0
        pe = (3 * bw * (YW + ov * (ch - 1)) + 3 * nv * (XW + ov * (bw - 1))
              + 2 * (YW + XW)) / 2.4
        # PSUM-drain work (Act+DVE only): copies + paired moves + abs
        nmv = -(-nv // 2) if CFG["pair"] else nv
        engAD = (3 * bw * (YW * 0.93 + 250)
                 + 3 * (nv * XW * 0.93 + nmv * 250)
                 + max(ch, bw) * ((YW + XW) * 0.6 + 250))
        # SBUF-side work (DVE 2x / Pool): mins + memsets
        engSB = max(ch, bw) * ((YW + XW) * 0.6 + 200) + 1000
        return read, pe, engAD, engSB

    def total(per_slot):
        read = sum(p[0] for p in per_slot)
        pe = sum(p[1] for p in per_slot)
        engAD = sum(p[2] for p in per_slot)
        engSB = sum(p[3] for p in per_slot)
        dma = read + 30106.0
        c = dma
        c += 2.0 * max(0.0, pe - 0.85 * dma)
        c += CFG.get("w_ad", 2.0) * max(
            0.0, (engAD + max(0.0, engSB - 0.8 * dma)) / 2.0
            - CFG.get("th_ad", 0.75) * dma)
        return c

    best_groups, best_cost = None, None
    for seed_kind in ("hw", "aspect"):
        if seed_kind == "hw":
            order = np.argsort(-h)
        else:
            order = np.argsort(-(nh * 512 + np.maximum(h, w)))
        groups = []
        for q in range(4):
            quart = order[q * 16:(q + 1) * 16]
            qs = quart[np.argsort(-w[quart])]
            groups += [qs[:8].copy(), qs[8:].copy()]
        rng = np.random.default_rng(0)
        per_slot = [proxy(g) for g in groups]
        best = total(per_slot)
        for _ in range(40000):
            s1, s2 = rng.integers(0, 8, 2)
            if s1 == s2:
                continue
            i1, i2 = rng.integers(0, 8, 2)
            g1, g2 = groups[s1], groups[s2]
            g1[i1], g2[i2] = g2[i2], g1[i1]
            o1, o2 = per_slot[s1], per_slot[s2]
            per_slot[s1] = proxy(g1)
            per_slot[s2] = proxy(g2)
            c = total(per_slot)
            if c <= best:
                best = c
            else:
                g1[i1], g2[i2] = g2[i2], g1[i1]
                per_slot[s1], per_slot[s2] = o1, o2
        if best_cost is None or best < best_cost:
            best_cost, best_groups = best, [g.copy() for g in groups]

    groups = best_groups
    sizes = [proxy(g)[0] + proxy(g)[1] for g in groups]
    order = list(np.argsort(sizes)[::-1])
    so = CFG.get("slot_order", "desc")
    if so == "small_first":
        order = [order[-1]] + order[:-1]
    elif so == "small_first2":
        order = [order[-1], order[-2]] + order[:-2]
    elif so == "interleave":
        a, b = order[:4], order[4:][::-1]
        order = [x for p in zip(b, a) for x in p]
    elif so == "smfirst_smlast":
        order = [order[-1], order[-2]] + order[:-3] + [order[-3]]
    elif so == "zigzag":
        rest = order[:-2]
        a, b = rest[:3], rest[3:]
        z = []
        for i in range(3):
            z.append(a[i])
            if i < len(b):
                z.append(b[i])
        order = [order[-1], order[-2]] + z
    elif so == "perm":
        # explicit permutation of the size-ranked slots (0 = biggest)
        order = [order[i] for i in CFG["perm"]]
    groups = [groups[i] for i in order]
    specs = tuple(_slot_spec([int(i) for i in g], boxes) for g in groups)
    return groups, specs


# ---------------------------------------------------------------------------
# layout bookkeeping shared by host and device

def _crop_rows(sp):
    """Packed row count: pad to 128*ch when the waste is tiny (single DMA),
    else keep RH rows (two DMAs)."""
    RH, RW, ch = sp[0], sp[1], sp[2]
    return 128 * ch if (128 * ch - RH) * 6 * RW <= 40000 else RH


def _layout(specs):
    crop_off = []
    o = 0
    for sp in specs:
        crop_off.append(o)
        o += _crop_rows(sp) * 3 * sp[1]
    crop_tot = o

    par_off = []    # (y_hi, x_hi, y_lo, x_lo) offsets per slot in parhl row
    o = 0
    for sp in specs:
        YW, XW = sp[6], sp[7]
        par_off.append((o, o + YW, o + YW + XW, o + 2 * YW + XW))
        o += 2 * (YW + XW)
    ones_off = o
    par_tot = o + 128

    # tent block offsets per slot: paired blocks [2, W] first, then singles
    tb_off = []
    mx = 0
    for sp in specs:
        ch, bw = sp[2], sp[3]
        ybl, xbl = sp[11], sp[14]
        m = min(ch, bw)
        o = 0
        offs_y, offs_x = [0] * ch, [0] * bw
        for t in range(m):
            wdt = ybl[t][1] - ybl[t][0]
            offs_y[t] = o
            offs_x[t] = o + wdt
            o += 2 * wdt
        for t in range(m, ch):
            offs_y[t] = o
            o += ybl[t][1] - ybl[t][0]
        for t in range(m, bw):
            offs_x[t] = o
            o += xbl[t][1] - xbl[t][0]
        tb_off.append((tuple(offs_y), tuple(offs_x)))
        mx = max(mx, o)
    return dict(crop_off=crop_off, crop_tot=crop_tot, par_off=par_off,
                ones_off=ones_off, par_tot=par_tot, tb_off=tb_off,
                tent_max=mx)


# ---------------------------------------------------------------------------
# device program

class _Balance:
    """Static greedy load balancer. Pool never touches PSUM (illegal), and
    abs only exists on Act (activation table)."""

    def __init__(self, nc):
        self.nc = nc
        self.load = {"act": 0.0, "dve": 0.0, "pool": 0.0}
        self.tabs_sb = None

    def _cost(self, eng, width, psum, poolmul, dve2x):
        if eng == "act":
            return width * 0.833 + 330
        if eng == "dve":
            return width * (0.521 if dve2x and not psum else 1.042) \
                + (205 if psum else 140)
        return width * 0.833 / (0.42 if poolmul else 0.6) + 240

    def pick(self, width, psum=False, poolmul=False, dve2x=False,
             allowed=("act", "dve", "pool")):
        if psum:
            allowed = tuple(e for e in allowed if e != "pool")
        eng = min(allowed,
                  key=lambda e: self.load[e] + self._cost(e, width, psum,
                                                          poolmul, dve2x))
        self.load[eng] += self._cost(eng, width, psum, poolmul, dve2x)
        return eng

    def copy(self, out, in_, width):
        eng = self.pick(width, psum=True)
        if eng == "act":
            self.nc.scalar.copy(out, in_)
        else:
            self.nc.vector.tensor_copy(out=out, in_=in_)

    def move127(self, out, in_, width, parts):
        eng = self.pick(width, psum=True)
        if eng == "act":
            self.nc.scalar.activation(out, in_,
                                      mybir.ActivationFunctionType.Identity,
                                      bias=self.tabs_sb[0:parts, 4:5],
                                      scale=1.0)
        else:
            self.nc.vector.tensor_scalar(out=out, in0=in_, scalar1=FILL,
                                         scalar2=None,
                                         op0=mybir.AluOpType.add)

    def absop(self, out, in_, bias_ap, width):
        # Abs exists only on the Act engine
        self.load["act"] += width * 0.833 + 330
        self.nc.scalar.activation(out, in_, mybir.ActivationFunctionType.Abs,
                                  bias=bias_ap, scale=-1.0)

    def minop(self, out, in_, width):
        eng = self.pick(width, psum=False, dve2x=True,
                        allowed=("dve", "pool"))
        e = self.nc.vector if eng == "dve" else self.nc.gpsimd
        e.tensor_scalar(out=out, in0=in_, scalar1=1.0, scalar2=0.0,
                        op0=mybir.AluOpType.subtract,
                        op1=mybir.AluOpType.min)


def _build(specs):
    lay = _layout(specs)
    nc = bacc.Bacc("TRN2", target_bir_lowering=False, debug=False)

    crops = nc.dram_tensor("crops", [lay["crop_tot"]], BF16,
                           kind="ExternalInput")
    parhl = nc.dram_tensor("parhl", [1, lay["par_tot"]], BF16,
                           kind="ExternalInput")
    tabs = nc.dram_tensor("tabs", [128, NTAB], F32, kind="ExternalInput")
    # y-major output: [slot, y, c, x]; host transposes back when unpacking.
    out = nc.dram_tensor("out", [BL, O, C, O], F32, kind="ExternalOutput")

    with tile.TileContext(nc) as tc:
        with (
            tc.tile_pool(name="const", bufs=1) as cpool,
            tc.tile_pool(name="crop", bufs=1) as crop_pool,
            tc.tile_pool(name="tent", bufs=CFG["tent_bufs"]) as tent_pool,
            tc.tile_pool(name="dtmp", bufs=CFG["dtmp_bufs"]) as dtmp_pool,
            tc.tile_pool(name="rt", bufs=CFG["rt_bufs"]) as rt_pool,
            tc.tile_pool(name="osb", bufs=CFG["osb_bufs"]) as out_pool,
            tc.tile_pool(name="ps1", bufs=CFG["ps1_bufs"], space="PSUM") as ps1,
            tc.tile_pool(name="ps2", bufs=CFG["ps2_bufs"], space="PSUM") as ps2,
        ):
            bal = _Balance(nc)

            parhl_sb = cpool.tile([1, lay["par_tot"]], BF16, tag="parhl")
            tabs_sb = cpool.tile([128, NTAB], F32, tag="tabs")
            bal.tabs_sb = tabs_sb
            c127 = cpool.tile([128, C, O], F32, tag="c127")
            nc.gpsimd.memset(c127[:, :, :], FILL)

            crop_sb = []
            if CFG.get("const_act", 0):
                # consts on the Act HWDGE queue: SP's first crop gen starts
                # immediately; const gens pipeline behind it on the shared
                # HWDGE device (their transfers are tiny)
                nc.scalar.dma_start(parhl_sb[:], parhl[:])
                nc.scalar.dma_start(tabs_sb[:], tabs[:])
            elif CFG.get("const_after", 1) < 0:
                nc.sync.dma_start(parhl_sb[:], parhl[:])
                nc.sync.dma_start(tabs_sb[:], tabs[:])
            for s, sp in enumerate(specs):
                if s == CFG.get("const_after", 1):
                    # consts ride behind the first crop read(s)
                    nc.sync.dma_start(parhl_sb[:], parhl[:])
                    nc.sync.dma_start(tabs_sb[:], tabs[:])
                RH, RW, ch = sp[0], sp[1], sp[2]
                t = crop_pool.tile([128, ch, C, RW], BF16, tag=f"crop{s}")
                crop_sb.append(t)
                co = lay["crop_off"][s]
                if _crop_rows(sp) == 128 * ch:
                    srcap = AP(tensor=crops, offset=co,
                               ap=[[C * RW, 128], [128 * C * RW, ch],
                                   [RW, C], [1, RW]])
                    nc.sync.dma_start(t[0:128, 0:ch, :, :], srcap)
                else:
                    full = RH // 128
                    rem = RH - 128 * full
                    if full:
                        srcap = AP(tensor=crops, offset=co,
                                   ap=[[C * RW, 128], [128 * C * RW, full],
                                       [RW, C], [1, RW]])
                        nc.sync.dma_start(t[0:128, 0:full, :, :], srcap)
                    if rem:
                        srcap = AP(tensor=crops,
                                   offset=co + full * 128 * C * RW,
                                   ap=[[C * RW, rem], [RW, C], [1, RW]])
                        nc.sync.dma_start(t[0:rem, full, :, :], srcap)

            state = {}

            def prep(s):
                _mark(nc, s, "prep")
                sp = specs[s]
                (RH, RW, ch, bw, nv, nx, YW, XW, Y0, X0, cs,
                 ybl, yreg, yspan, xbl, xreg, xspan) = sp
                ohy, ohx, oly, olx = lay["par_off"][s]
                oo = lay["ones_off"]

                pb = ps2.tile([128, 2, 512], F32, tag="pm2")
                for part, (W_, ohi, olo) in enumerate(
                        ((YW, ohy, oly), (XW, ohx, olx))):
                    nc.tensor.matmul(pb[:, part, 0:W_],
                                     parhl_sb[0:1, oo:oo + 128],
                                     parhl_sb[0:1, ohi:ohi + W_],
                                     start=True, stop=False)
                    nc.tensor.matmul(pb[:, part, 0:W_],
                                     parhl_sb[0:1, oo:oo + 128],
                                     parhl_sb[0:1, olo:olo + W_],
                                     start=False, stop=True)

                _mark(nc, s, "tents")
                tent = tent_pool.tile([128, lay["tent_max"]], BF16, tag="tent")
                offs_y, offs_x = lay["tb_off"][s]
                m = min(ch, bw)
                for t in range(m):
                    lo, hi = ybl[t]
                    wdt = hi - lo
                    dtmp = dtmp_pool.tile([128, 2, 512], F32, tag="dtmp")
                    bal.absop(dtmp[:, :, 0:wdt], pb[:, :, lo:lo + wdt],
                              tabs_sb[:, t:t + 1], 2 * wdt)
                    bal.minop(
                        tent[:, offs_y[t]:offs_y[t] + 2 * wdt].rearrange(
                            "p (a b) -> p a b", a=2),
                        dtmp[:, :, 0:wdt], 2 * wdt)
                for part, rng_, blocks, offs in (
                        (0, range(m, ch), ybl, offs_y),
                        (1, range(m, bw), xbl, offs_x)):
                    for t in rng_:
                        lo, hi = blocks[t]
                        wdt = hi - lo
                        dtmp = dtmp_pool.tile([128, 2, 512], F32, tag="dtmp")
                        bal.absop(dtmp[:, 0, 0:wdt],
                                  pb[:, part, lo:lo + wdt],
                                  tabs_sb[:, t:t + 1], wdt)
                        bal.minop(tent[:, offs[t]:offs[t] + wdt],
                                  dtmp[:, 0, 0:wdt], wdt)
                state[s] = tent

            def stage1(s):
                _mark(nc, s, "st1")
                sp = specs[s]
                (RH, RW, ch, bw, nv, nx, YW, XW, Y0, X0, cs,
                 ybl, yreg, yspan, xbl, xreg, xspan) = sp
                tent = state[s]
                offs_y, offs_x = lay["tb_off"][s]
                crop = crop_sb[s]
                ylo, yhi = yspan
                rt = rt_pool.tile([128, C, bw, O], BF16, tag="rt")
                if ylo > 0:
                    nc.gpsimd.memset(rt[:, :, :, 0:ylo], 0.0)
                if yhi < YW:
                    nc.gpsimd.memset(rt[:, :, :, yhi:YW], 0.0)
                for c in range(C):
                    k2 = 0
                    while k2 < bw:
                        npair = 2 if (CFG["pair1"] and k2 + 1 < bw) else 1
                        kwmax = min(128, RW - 128 * k2)
                        pmm = ps1.tile(
                            [128, 2 if CFG["pair1"] else 1, 512], F32,
                            tag="pmm")
                        for kk in range(npair):
                            kw = min(128, RW - 128 * (k2 + kk))
                            for (r0, r1, pieces) in yreg:
                                for i, t in enumerate(pieces):
                                    rows = min(128, RH - 128 * t)
                                    lo = ybl[t][0]
                                    toff = offs_y[t]
                                    nc.tensor.matmul(
                                        pmm[0:kw, kk, r0:r1],
                                        crop[0:rows, t, c,
                                             128 * (k2 + kk):
                                             128 * (k2 + kk) + kw],
                                        tent[0:rows,
                                             toff + r0 - lo:toff + r1 - lo],
                                        start=(i == 0),
                                        stop=(i == len(pieces) - 1))
                        bal.copy(rt[0:kwmax, c, k2:k2 + npair, ylo:yhi],
                                 pmm[0:kwmax, 0:npair, ylo:yhi],
                                 npair * (yhi - ylo))
                        k2 += npair
                state[s] = (tent, rt)

            def stage2(s):
                _mark(nc, s, "st2")
                sp = specs[s]
                (RH, RW, ch, bw, nv, nx, YW, XW, Y0, X0, cs,
                 ybl, yreg, yspan, xbl, xreg, xspan) = sp
                tent, rt = state.pop(s)
                offs_y, offs_x = lay["tb_off"][s]
                if Y0 and CFG.get("bands_late", 1):
                    for row0 in (0, O - Y0):
                        bdst = AP(tensor=out, offset=(s * O + row0) * C * O,
                                  ap=[[C * O, Y0], [1, C * O]])
                        nc.gpsimd.dma_start(bdst, c127[0:Y0, :, :])
                xlo, xhi = xspan
                osb = out_pool.tile([128, 3, C, O], F32, tag="osb")
                if X0 + xlo > 0:
                    nc.gpsimd.memset(osb[0:cs, 0:nv, :, 0:X0 + xlo], FILL)
                if X0 + xhi < O:
                    nc.gpsimd.memset(osb[0:cs, 0:nv, :, X0 + xhi:O], FILL)
                # pairing plan over the (c, j) grid: j-pairs per channel,
                # then cross-channel pairs for a leftover j column
                plan = []
                if CFG["pair"]:
                    for c in range(C):
                        j = 0
                        while j + 1 < nv:
                            plan.append(((c, j), (c, j + 1)))
                            j += 2
                    if nv % 2 == 1:
                        jl = nv - 1
                        plan.append(((0, jl), (1, jl)))
                        plan.append(((2, jl),))
                else:
                    plan = [((c, j),) for c in range(C) for j in range(nv)]
                for cells in plan:
                    pm2 = ps2.tile([128, 2, 512], F32, tag="pm2")
                    for idx, (c, j) in enumerate(cells):
                        for (x0, x1, pieces) in xreg:
                            for i, k2 in enumerate(pieces):
                                kw = min(128, RW - 128 * k2)
                                lo = xbl[k2][0]
                                toff = offs_x[k2]
                                nc.tensor.matmul(
                                    pm2[0:cs, idx, x0:x1],
                                    rt[0:kw, c, k2, cs * j:cs * j + cs],
                                    tent[0:kw, toff + x0 - lo:toff + x1 - lo],
                                    start=(i == 0),
                                    stop=(i == len(pieces) - 1))
                    (c0, j0) = cells[0]
                    if len(cells) == 1:
                        dstap = osb[0:cs, j0, c0, X0 + xlo:X0 + xhi]
                    elif cells[1][0] == c0:       # j-pair
                        dstap = osb[0:cs, j0:j0 + 2, c0, X0 + xlo:X0 + xhi]
                    else:                          # c-pair
                        dstap = osb[0:cs, j0, c0:c0 + 2, X0 + xlo:X0 + xhi]
                    bal.move127(dstap, pm2[0:cs, 0:len(cells), xlo:xhi],
                                len(cells) * (xhi - xlo), cs)
                _mark(nc, s, "outdma")
                if CFG.get("split_out", 0) and nv > 1:
                    for j in range(nv):
                        dst = AP(tensor=out,
                                 offset=(s * O + Y0 + j * cs) * C * O,
                                 ap=[[C * O, cs], [1, C * O]])
                        nc.sync.dma_start(dst, osb[0:cs, j, :, :])
                else:
                    dst = AP(tensor=out, offset=(s * O + Y0) * C * O,
                             ap=[[C * O, cs], [cs * C * O, nv], [1, C * O]])
                    nc.sync.dma_start(dst, osb[0:cs, 0:nv, :, :])

            if not CFG.get("bands_late", 1):
                for s, sp in enumerate(specs):
                    Y0b = sp[8]
                    if Y0b:
                        for row0 in (0, O - Y0b):
                            bdst = AP(tensor=out,
                                      offset=(s * O + row0) * C * O,
                                      ap=[[C * O, Y0b], [1, C * O]])
                            nc.gpsimd.dma_start(bdst, c127[0:Y0b, :, :])

            PA, SA = CFG["PA"], CFG["SA"]
            ramp = CFG.get("ramp", 0)
            due = {}
            for s in range(BL):
                lag = max(1, min(s, SA)) if ramp else SA
                due.setdefault(s + PA + lag, []).append(s)
            for i in range(BL + PA + SA + 1):
                if CFG["st2_first"]:
                    for s in due.get(i, []):
                        stage2(s)
                if i < BL:
                    prep(i)
                if 0 <= i - PA < BL:
                    stage1(i - PA)
                if not CFG["st2_first"]:
                    for s in due.get(i, []):
                        stage2(s)

    nc.compile()
    return nc


# ---------------------------------------------------------------------------
# host-side packing

def _host_params(images, boxes, specs):
    """Per-core host prep. images: [BL,3,768,768] f32, boxes: [BL,4] i32."""
    import ml_dtypes
    bf16 = ml_dtypes.bfloat16
    lay = _layout(specs)

    crops = np.zeros(lay["crop_tot"], dtype=bf16)
    tabs = np.zeros((128, NTAB), dtype=np.float32)
    tabs[:, 0:4] = (np.arange(128)[:, None]
                    + 128 * np.arange(4)[None, :]).astype(np.float32)
    tabs[:, 4] = FILL
    par_hi = np.zeros((1, lay["par_tot"]), dtype=bf16)
    par_hi[0, lay["ones_off"]:] = bf16(1.0)

    for s, sp in enumerate(specs):
        (RH, RW, ch, bw, nv, nx, YW, XW, Y0, X0, cs, *_rest) = sp
        xb, yb, wb, hb = (int(v) for v in boxes[s])
        nh, nw, pt, pl = _img_geom(hb, wb)

        cw = images[s, :, yb:yb + hb, xb:xb + wb] - np.float32(FILL)
        co = lay["crop_off"][s]
        rows = _crop_rows(sp)
        dst = crops[co:co + rows * 3 * RW].reshape(rows, 3, RW)
        dst[0:hb, :, 0:wb] = cw.transpose(1, 0, 2).astype(bf16)

        syc, _ = _axis_coords(pt, nh, hb, Y0, YW)
        sxc, _ = _axis_coords(pl, nw, wb, X0, XW)
        ohy, ohx, oly, olx = lay["par_off"][s]
        for off_hi, off_lo, arr in ((ohy, oly, syc), (ohx, olx, sxc)):
            hi = arr.astype(bf16)
            lo = (arr - hi.astype(np.float32)).astype(bf16)
            par_hi[0, off_hi:off_hi + len(arr)] = hi
            par_hi[0, off_lo:off_lo + len(arr)] = lo

    return dict(crops=crops, parhl=par_hi, tabs=tabs)


def kernel(images: np.ndarray, boxes: np.ndarray) -> np.ndarray:
    global LAST_RESULT
    boxes = np.asarray(boxes)
    images = np.asarray(images, dtype=np.float32)
    groups, specs = _plan(boxes)
    if specs not in _CACHED:
        _CACHED[specs] = _build(specs)
    nc = _CACHED[specs]

    in_maps = []
    for m in range(N_CORES):
        idx = [int(groups[s][m]) for s in range(BL)]
        in_maps.append(_host_params(images[idx], boxes[idx], specs))
    res = run_bass_kernel_spmd(nc, in_maps, core_ids=list(range(N_CORES)))
    LAST_RESULT = res
    full = np.empty((B, C, O, O), np.float32)
    for m in range(N_CORES):
        for s in range(BL):
            full[int(groups[s][m])] = res.results[m]["out"][s].transpose(1, 0, 2)
    return full
